# revision 18
# baseline (speedup 1.0000x reference)
"""DetConB loss (nn_DetConBLoss) on 8 TRN2 NeuronCores via Bass/Tile.

v2 strategy (data-parallel over batch, targets replicated):
  - Host: l2-normalize preds/targets in f32, flatten to (4096, 256),
    transpose to (d, rows), cast fp8. Core c owns pred rows
    [c*512, (c+1)*512). Each core receives the full targets with columns
    rolled by c*512 so its own-image diagonal band sits at a fixed,
    compile-time-constant column range (the program is SPMD-identical).
  - Device (per core), per (pred, target) combination u of 4, the
    denominator sums S[p] = sum_t exp(scale * <p, t>) are computed with
    the logits split by target range between the two PSUM consumers:
    * ACT range, targets [0, TA) (normal layout): per 128-pred tile,
      DoubleRow fp8 matmuls put [128 preds, 512-target] slabs into
      1-bank PSUM slots; ONE ScalarE exp over a strided multi-bank AP
      (N=2048 amortizes the ~480ns per-instruction overhead) with the
      ACT accumulator producing the per-pred row sums for free (f32,
      no extra PE work, no fp8 rounding of the summands).
    * DVE range, targets [TA, 4096) (transposed layout): 128-target
      blocks become PSUM partitions ([128 targets, 512 preds] per
      bank, 2 banks per chunk); DVE applies a Schraudolph uint8
      construct (y = x*SA8 + SB8, bitcast fp8e4) and a ones-vector
      DoubleRow matmul contracts the 256 target partitions on the
      TensorE, accumulating per-pred partials in PSUM.
  - PSUM: 6 fungible 1-bank slots feed both consumers (model-driven
    compile-time bank allocator; Tile's bank-aware overlap tracking
    provides the hazards). Banks 6/7 hold the two per-px DVE partial
    accumulators: the loss only uses S[p1] = S[0]+S[2] and
    S[p2] = S[1]+S[3], so combos sharing a pred view accumulate into
    one bank across their whole lifetime (single matmul accumulation
    group per bank, start on the first mm2, stop on the last).
  - Host: the 16x16 own-image diagonal dot blocks (recomputed from the
    same fp8 inputs), masks from the roi indices, positive-pair sums,
    the -inf masking correction, log, and the final mean.
"""
import numpy as np
import ml_dtypes

import concourse.bacc as bacc
import concourse.mybir as mybir
import concourse.tile as tile
from concourse.bass_utils import run_bass_kernel_spmd

TEMP = 0.1
EPS = 1e-11
SCALE = float(np.float32(1.0 / (TEMP + EPS)))
NCORES = 8
B, N, D = 256, 16, 256
R = B * N          # 4096 flat rows
RPC = R // NCORES  # 512 rows per core
BF16 = mybir.dt.bfloat16
FP8 = mybir.dt.float8e4
U8 = mybir.dt.uint8
NPFP8 = ml_dtypes.float8_e4m3
F32 = mybir.dt.float32

# Schraudolph fast-exp, e4m3 flavor for the DVE blocks:
# exp(x) ~= bitcast_fp8e4(uint8(x*SA8 + SB8)), x = scale*logit.
SA8 = float(np.float32(8.0 / np.log(2.0)))
SB8 = float(np.float32(7 * 8 - 486411.0 / 2**20))

# Per-combo ACT target count (uniform across the 4 pred tiles: the DVE's
# transposed chunks always span all 512 preds, so the split must agree).
# Combo 1 is ACT-heavy: it runs while the t2 input stream is in flight,
# when the PE has the least fp8 fetch bandwidth to spare. The diagonal
# band (cols [0, 512)) is always inside the ACT range, so the host's
# np.exp-based -inf correction matches the device's ACT-range math.
TA_PER_COMBO = [2048, 2048, 2048, 2048]
NBANKS = 6                # fungible logit banks (banks 6,7 = DVE accs)

# Schedule-shape knobs (searched offline against sched_sim's event model).
# Uniform 1024-elem (2-bank) chunks for both consumers: the 6-bank pool
# supports full double-buffering of both streams only at this
# granularity, and TA=2048 balances ACT vs DVE work within every combo
# (larger chunks amortize per-instruction overhead better but stall on
# PSUM banks). The loss only ever uses S[0]+S[2] and S[1]+S[3] (same
# pred view), so the DVE partials of combos sharing a px accumulate into
# ONE bank each: bank 6 for p1 (combos 0,2), bank 7 for p2 (combos 1,3),
# both at matmul output partition base 0 (DoubleRow matmuls cannot
# address other partition bases -- s3d3_mm_valid_dst_partition).
SCHED_CFG = {
    'ta': TA_PER_COMBO,
    'a_size': 1024,
}

# nominal per-instruction times (ns) for the model-driven bank allocator
_MM_NS = 242
_ACT_GAP = 190
_DVE_GAP = 92


def _act_ns(n):
    return (n + 352) / 1.2 + _ACT_GAP


def _dve_ns(n):
    return (n + 120) / 0.96 + _DVE_GAP


def build_schedule(cfg=None):
    """Compile-time chunk schedule + PSUM bank assignment.

    Chunk dicts:
      {'kind':'A', 'u', 'tile', 't0', 'size' (1024|2048), 'banks', 'col'}
      {'kind':'V', 'u', 't0', 'banks': [b0, b1], 'first', 'last'}
    """
    cfg = cfg or SCHED_CFG
    a_size = cfg['a_size']
    chunks = []
    acol = 0
    for u in range(4):
        ta = cfg['ta'][u]
        aq = []
        if u == 0:
            # stream-aligned: all tiles at targets [0,1024) first, then
            # the [1024,2048) chunks -- matches the input-segment order
            for t0 in range(0, ta, a_size):
                for t in range(4):
                    aq.append({'kind': 'A', 'u': u, 'tile': t, 't0': t0,
                               'size': a_size})
        else:
            for t in range(4):
                t0 = 0
                while t0 < ta:
                    size = a_size if ta - t0 >= a_size else (ta - t0)
                    aq.append({'kind': 'A', 'u': u, 'tile': t, 't0': t0,
                               'size': size})
                    t0 += size
        vq = [{'kind': 'V', 'u': u, 't0': t0}
              for t0 in range(ta, 4096, 256)]
        # proportional interleave of the two queues; combo 0 front-loads
        # three A-chunks so the ACT stream starts while the V-range target
        # columns (loaded later in the stream-aligned order) arrive.
        seq = []
        na, nv = len(aq), len(vq)
        ia = iv = 0
        hold_a = 2 if u == 3 else 0    # combo 3 ends on A-chunks
        if u == 0:
            while ia < 3:
                seq.append(aq[ia]); ia += 1
        while ia < na - hold_a or iv < nv:
            if iv >= nv or (ia < na - hold_a
                            and (ia - (3 if u == 0 else 0)) * nv
                            <= iv * (na - hold_a - (3 if u == 0 else 0))):
                seq.append(aq[ia]); ia += 1
            else:
                seq.append(vq[iv]); iv += 1
        while ia < na:
            seq.append(aq[ia]); ia += 1
        for ch in seq:
            if ch['kind'] == 'A':
                ch['col'] = acol
                acol += 1
        chunks.extend(seq)
    assert acol <= 32, acol
    # V accumulation groups are per-px (combos {0,2} -> bank 6, {1,3} -> 7)
    for px in range(2):
        vs = [c for c in chunks if c['kind'] == 'V' and c['u'] % 2 == px]
        for c in vs:
            c['first'] = c['last'] = False
        vs[0]['first'] = True
        vs[-1]['last'] = True
    _assign_model_driven(chunks)
    return chunks


def _bank_sets(n):
    """All constant-stride n-bank sets within banks [0, NBANKS)."""
    sets = []
    for s in range(1, NBANKS):
        for b in range(NBANKS - (n - 1) * s):
            sets.append(tuple(b + k * s for k in range(n)))
    return sets


def _assign_model_driven(chunks):
    """Pick each chunk's banks to minimize its predicted matmul start
    time under a nominal-latency event model of the three engine queues
    (in-order PE / ACT / DVE, bank WAR hazards, mm2 pending-3 FIFO),
    mirroring build_nc's emission order exactly."""
    set_cache = {}
    bank_free = [0.0] * NBANKS   # when the bank's previous consumer is done
    pe_t = act_t = dve_t = 0.0
    pend = []                    # dve_done times, mm2 FIFO model

    for ch in chunks:
        nb = (ch['size'] // 512) if ch['kind'] == 'A' else 2
        if nb not in set_cache:
            set_cache[nb] = _bank_sets(nb)
        best, best_cost = None, None
        for cs in set_cache[nb]:
            cost = (max(bank_free[b] for b in cs),
                    sum(bank_free[b] for b in cs))
            if best is None or cost < best_cost:
                best, best_cost = cs, cost
        banks = list(best)
        ch['banks'] = banks
        # simulate: MMs wait for bank frees (in-order PE head-of-line)
        for k in range(nb):
            pe_t = max(pe_t, bank_free[banks[k]]) + _MM_NS
        if len(pend) == 3:                      # mm2 flush slot
            pe_t = max(pe_t, pend.pop(0)) + _MM_NS
        if ch['kind'] == 'A':
            act_t = max(act_t, pe_t) + _act_ns(ch['size'])
            done = act_t
        else:
            dve_t = max(dve_t, pe_t) + _dve_ns(1024)
            done = dve_t
            pend.append(done)
        for b in banks:
            bank_free[b] = done
    return chunks


def build_nc():
    """Build + schedule + compile the SPMD per-core Bass program."""
    schedule = build_schedule()

    nc = bacc.Bacc("TRN2", target_bir_lowering=False, debug=False,
                   num_devices=NCORES)

    # p layout [128, 2048]: k0 at cols [0,512), k1 at [1024,1536), rest pad
    # (the 512-byte gap keeps the DoubleRow k-pair fetch off a single SBUF
    # line; adjacent k-runs halve the PE's fp8 stream rate).
    p_dram = [nc.dram_tensor(f"p{i + 1}t", [128, 4 * RPC], FP8,
                             kind="ExternalInput") for i in range(2)]
    t_dram = [nc.dram_tensor(f"t{i + 1}t", [D, R], FP8, kind="ExternalInput")
              for i in range(2)]
    dsumA = nc.dram_tensor("dsumA", [128, 32], F32, kind="ExternalOutput")
    dsumV = nc.dram_tensor("dsumV", [32, 2 * RPC], F32, kind="ExternalOutput")

    with tile.TileContext(nc) as tc:
        with (
            tc.tile_pool(name="const", bufs=1) as const_pool,
            tc.tile_pool(name="psum", bufs=1, space="PSUM") as psum_pool,
            tc.tile_pool(name="scratch", bufs=2) as scratch_pool,
        ):
            t_sb = [const_pool.tile([128, 2 * R], FP8, name=f"t_sb{i}", tag=f"t{i}")
                    for i in range(2)]
            p_sb = [const_pool.tile([128, 4 * RPC], FP8, name=f"p_sb{i}", tag=f"p{i}")
                    for i in range(2)]

            warm = const_pool.tile([128, 2], F32, name="warm", tag="warm")
            zbias = const_pool.tile([128, 1], F32, name="zbias", tag="zbias")
            ones8 = const_pool.tile([128, 64], FP8, name="ones8", tag="ones8")
            accA = const_pool.tile([128, 32], F32, name="accA", tag="accA")
            dsb = const_pool.tile([128, 2 * RPC], F32, name="dsb", tag="dsb")
            # dead-store target for the ACT exp main output (bf16: no fp8
            # saturation in case hardware accumulates post-cast values)
            aout = const_pool.tile([128, 2048], BF16, name="aout", tag="aout")

            # Input DMAs. The loads gating the first matmuls are spread
            # across descriptor-generation engines and emitted first.
            def load_t_cols(tsel, k, c0, c1, eng=None):
                (eng or nc.sync).dma_start(
                    out=t_sb[tsel][:, k * R + c0: k * R + c1],
                    in_=t_dram[tsel][k * 128:(k + 1) * 128, c0:c1])

            def load_t(tsel, k, g):
                load_t_cols(tsel, k, g * 2048, (g + 1) * 2048)

            def load_p(px, eng):
                # only the two used 512-col halves, not the padding
                for k in range(2):
                    eng.dma_start(
                        out=p_sb[px][:, k * 2 * RPC:k * 2 * RPC + RPC],
                        in_=p_dram[px].ap()[:, k * 2 * RPC:k * 2 * RPC + RPC])

            # Segments ordered to match combo 0's consumption: the ACT
            # range [0,2048) first, then the DVE range [2048,4096), each
            # k-half on a different descriptor-generation engine so the
            # serial ~0.65us DIRECT2D gens pipeline across sync (HWDGE),
            # scalar (HWDGE) and gpsimd (SWDGE).
            load_t_cols(0, 0, 0, 1024, eng=nc.gpsimd)
            load_t_cols(0, 1, 0, 1024, eng=nc.scalar)
            load_p(0, nc.sync)
            load_t_cols(0, 0, 2048, 3072, eng=nc.gpsimd)
            load_t_cols(0, 1, 2048, 3072, eng=nc.scalar)
            load_t_cols(0, 0, 1024, 2048, eng=nc.sync)
            load_t_cols(0, 1, 1024, 2048, eng=nc.scalar)
            load_p(1, nc.sync)
            load_t_cols(0, 0, 3072, 4096, eng=nc.gpsimd)
            load_t_cols(0, 1, 3072, 4096, eng=nc.sync)

            nc.vector.memset(warm, 0.0)
            # Explicit zero-bias AP: a float bias would be lowered through the
            # const-AP machinery, whose TENSOR_LOAD sits in the preamble.
            nc.vector.memset(zbias, 0.0)
            # 32 identical ones columns: a DoubleRow LDWEIGHTS with a single
            # weight column fails the compiler's ISA check, so the sum
            # matmul produces 32 duplicate rows (only row 0 is read).
            nc.vector.memset(ones8, 1.0)
            # Warm the exp table set during the input-DMA window so the first
            # real ACTIVATE does not pay the ~2.7us ACT_TABLE_LOAD.
            nc.scalar.activation(warm, warm,
                                 mybir.ActivationFunctionType.Exp, bias=zbias)

            rhs3 = [t_sb[i].rearrange("p (k c) -> p k c", k=2) for i in range(2)]
            lhs3 = [p_sb[i].rearrange("p (k c) -> p k c", k=2) for i in range(2)]
            onesT = ones8.rearrange("p (k m) -> p k m", k=2)

            # One big PSUM tile: banks 0..6 = fungible logit slots,
            # bank 7 = the four packed DVE-partial accumulators.
            big = psum_pool.tile([128, 4096], F32, name="big", tag="big")
            bigb = big.rearrange("p (b c) -> p b c", c=512)   # [128, 8, 512]
            acc = [big[:, 6 * 512: 7 * 512], big[:, 7 * 512: 8 * 512]]

            aoutb = aout.rearrange("p (b c) -> p b c", c=512)

            def chunk_ap(banks):
                """[128, n, 512] AP over a constant-stride bank set."""
                bs = sorted(banks)
                s = bs[1] - bs[0]
                assert all(bs[k + 1] - bs[k] == s for k in range(len(bs) - 1))
                return bigb[:, bs[0]:bs[-1] + 1:s, :]

            # mm2 pending queue: a V-chunk's sum matmul is emitted three
            # chunks later so the in-order PE queue never head-blocks
            # waiting for the DVE's exp data. FIFO order also guarantees
            # combo u's accumulation group closes before combo u+1 opens.
            pending = []

            def flush_one():
                ch, sch = pending.pop(0)
                px = ch['u'] % 2
                nc.tensor.matmul(
                    acc[px][0:32, :],
                    onesT,
                    sch.bitcast(FP8).rearrange(
                        "p (k c) -> p k c", k=2)[:, :, 0:RPC],
                    start=ch['first'],
                    stop=ch['last'],
                    perf_mode=mybir.MatmulPerfMode.DoubleRow)
                if ch['last']:
                    # bounce this px's partials out right away on the DVE
                    # (the less-loaded consumer); px0 closes during combo
                    # 3, overlapping its copy with compute
                    nc.vector.tensor_scalar(
                        dsb[0:32, px * RPC:(px + 1) * RPC],
                        acc[px][0:32, :], 0.0, 0.0,
                        op0=mybir.AluOpType.add,
                        op1=mybir.AluOpType.bypass)
                    nc.sync.dma_start(
                        out=dsumV.ap()[:, px * RPC:(px + 1) * RPC],
                        in_=dsb[0:32, px * RPC:(px + 1) * RPC])

            t2_half = 0
            nchunk = 0
            for ch in schedule:
                u = ch['u']
                tsel, px = u // 2, u % 2
                nchunk += 1
                # t2 is first needed at combo 2 (~half-way). Its 1 MB of
                # DMA traffic steals fp8 fetch bandwidth from the PE, so
                # stagger it in two anchored halves through combos 0-1
                # (a 1-col memset anchor on the DVE queue orders each half
                # after the corresponding point of the consumer stream).
                if (t2_half == 0 and nchunk == 10) or \
                   (t2_half == 1 and nchunk == 22):
                    g = t2_half
                    for k in range(2):
                        nc.vector.memset(
                            t_sb[1][:, k * R + g * 2048:
                                    k * R + g * 2048 + 1], 0.0)
                    for k in range(2):
                        load_t(1, k, g)
                    t2_half += 1
                if ch['kind'] == 'A':
                    # normal layout: one matmul per bank; weights are the
                    # pred tile, moving data the target columns.
                    ti = ch['tile']
                    nmm = ch['size'] // 512
                    for s in range(nmm):
                        t0 = ch['t0'] + 512 * s
                        nc.tensor.matmul(
                            bigb[:, ch['banks'][s], :],
                            lhs3[px][:, :, ti * 128:(ti + 1) * 128],
                            rhs3[tsel][:, :, t0:t0 + 512],
                            start=True, stop=True,
                            perf_mode=mybir.MatmulPerfMode.DoubleRow)
                    if len(pending) == 3:
                        flush_one()
                    # one ScalarE exp over all banks of the chunk, with the
                    # fused accumulator writing the row sums to an accA col.
                    nb = ch['size'] // 512
                    nc.scalar.activation(
                        aoutb[:, 0:nb, :], chunk_ap(ch['banks']),
                        mybir.ActivationFunctionType.Exp,
                        bias=zbias, scale=SCALE,
                        accum_out=accA[:, ch['col']:ch['col'] + 1])
                else:
                    # transposed layout: 2 matmuls, target blocks as PSUM
                    # partitions; banks may be non-adjacent.
                    for h in range(2):
                        tc0 = ch['t0'] + 128 * h
                        nc.tensor.matmul(
                            bigb[:, ch['banks'][h], :],
                            rhs3[tsel][:, :, tc0:tc0 + 128],
                            lhs3[px][:, :, 0:RPC],
                            start=True, stop=True,
                            perf_mode=mybir.MatmulPerfMode.DoubleRow)
                    if len(pending) == 3:
                        flush_one()
                    pin = chunk_ap(ch['banks'])
                    sch = scratch_pool.tile([128, 2048], U8,
                                            name="sch", tag="sch", bufs=6)
                    pout = sch.rearrange("p (k c) -> p k c", k=2)[:, :, 0:RPC]
                    nc.vector.tensor_scalar(
                        pout, pin, SA8 * SCALE, SB8,
                        op0=mybir.AluOpType.mult,
                        op1=mybir.AluOpType.add)
                    pending.append((ch, sch))
            while pending:
                flush_one()

            nc.sync.dma_start(out=dsumA.ap(), in_=accA)

    nc.compile()
    return nc, schedule


_NC = None


def _get_nc():
    global _NC
    if _NC is None:
        _NC = build_nc()
    return _NC


def _l2norm(x):
    return x / np.linalg.norm(x, axis=-1, keepdims=True)


def _dev_p_layout(pt):
    # pt: [D=256, RPC] fp8 -> [128, 2048] with k0 at [0,512), k1 at [1024,1536)
    out = np.zeros((128, 4 * RPC), NPFP8)
    out[:, 0:RPC] = pt[0:128]
    out[:, 2 * RPC:3 * RPC] = pt[128:256]
    return out


def host_prep(pred1, pred2, target1, target2):
    p1t = _l2norm(np.asarray(pred1, np.float32)).reshape(R, D).T.astype(NPFP8)
    p2t = _l2norm(np.asarray(pred2, np.float32)).reshape(R, D).T.astype(NPFP8)
    t1t = _l2norm(np.asarray(target1, np.float32)).reshape(R, D).T.astype(NPFP8)
    t2t = _l2norm(np.asarray(target2, np.float32)).reshape(R, D).T.astype(NPFP8)
    # Raw own-image diagonal dot blocks (b, n, m), fp8-quantized operands in
    # f32 — the same products the device computes, ~0.4% of total FLOPs.
    pf = [p1t.T.astype(np.float32).reshape(B, N, D),
          p2t.T.astype(np.float32).reshape(B, N, D)]
    tf = [t1t.T.astype(np.float32).reshape(B, N, D),
          t2t.T.astype(np.float32).reshape(B, N, D)]
    diag = [[np.einsum('bnd,bmd->bnm', pf[px], tf[ts]).astype(np.float32)
             for ts in range(2)] for px in range(2)]
    in_maps = []
    for c in range(NCORES):
        r0 = c * RPC
        in_maps.append({
            "p1t": _dev_p_layout(p1t[:, r0:r0 + RPC]),
            "p2t": _dev_p_layout(p2t[:, r0:r0 + RPC]),
            "t1t": np.ascontiguousarray(np.concatenate([t1t[:, r0:], t1t[:, :r0]], axis=1)),
            "t2t": np.ascontiguousarray(np.concatenate([t2t[:, r0:], t2t[:, :r0]], axis=1)),
        })
    return in_maps, diag


def host_post(results, diag, pind1, pind2, tind1, tind2):
    _, schedule = _get_nc()
    S = np.zeros((2, R), np.float64)  # per px (p1/p2), per global pred row
    for c, res in enumerate(results):
        r0 = c * RPC
        dA = np.asarray(res["dsumA"]).astype(np.float64)   # [128, 32]
        dV = np.asarray(res["dsumV"]).astype(np.float64)   # [128, 1024]
        for ch in schedule:
            if ch['kind'] == 'A':
                px, ti = ch['u'] % 2, ch['tile']
                S[px, r0 + ti * 128: r0 + (ti + 1) * 128] += dA[:, ch['col']]
        for px in range(2):
            S[px, r0:r0 + RPC] += dV[0, px * RPC:(px + 1) * RPC]
    sc = np.float32(SCALE)
    D_aa = sc * diag[0][0]
    D_ab = sc * diag[0][1]
    D_ba = sc * diag[1][0]
    D_bb = sc * diag[1][1]

    f32 = np.float32
    pind1, pind2 = np.asarray(pind1), np.asarray(pind2)
    tind1, tind2 = np.asarray(tind1), np.asarray(tind2)
    same_aa = (pind1[:, :, None] == tind1[:, None, :]).astype(f32)
    same_ab = (pind1[:, :, None] == tind2[:, None, :]).astype(f32)
    same_ba = (pind2[:, :, None] == tind1[:, None, :]).astype(f32)
    same_bb = (pind2[:, :, None] == tind2[:, None, :]).astype(f32)

    S0 = S[0].reshape(B, N)  # p1: vs t1 + vs t2
    S1 = S[1].reshape(B, N)  # p2
    corr0 = (same_aa * np.exp(D_aa.astype(np.float64))).sum(-1)
    corr1 = (same_bb * np.exp(D_bb.astype(np.float64))).sum(-1)
    lse0 = np.log(S0 - corr0)
    lse1 = np.log(S1 - corr1)

    num_pos0 = same_ab.sum(-1)
    num_pos1 = same_ba.sum(-1)
    pos_sum0 = (same_ab * D_ab).sum(-1)
    pos_sum1 = (same_ba * D_ba).sum(-1)

    area0 = (pind1[:, :, None] == pind1[:, None, :]).astype(f32).sum(-1)
    area1 = (pind2[:, :, None] == pind2[:, None, :]).astype(f32).sum(-1)
    w0 = (num_pos0 > 0.001).astype(f32) / area0
    w1 = (num_pos1 > 0.001).astype(f32) / area1

    ce0 = -w0 * (pos_sum0 - num_pos0 * lse0) / np.maximum(num_pos0, 1.0)
    ce1 = -w1 * (pos_sum1 - num_pos1 * lse1) / np.maximum(num_pos1, 1.0)
    return np.float32(ce0.mean() + ce1.mean())


def run_hw(inputs, trace=False):
    nc, _ = _get_nc()
    in_maps, diag = host_prep(inputs["pred1"], inputs["pred2"],
                              inputs["target1"], inputs["target2"])
    last_err = None
    for attempt in range(3):
        try:
            res = run_bass_kernel_spmd(nc, in_maps,
                                       core_ids=list(range(NCORES)),
                                       trace=trace)
            break
        except Exception as e:  # transient NRT device errors recover on retry
            last_err = e
            import time
            time.sleep(20 * (attempt + 1))
    else:
        raise last_err
    loss = host_post(res.results, diag, inputs["pind1"], inputs["pind2"],
                     inputs["tind1"], inputs["tind2"])
    return loss, res


def kernel(**inputs):
    loss, _ = run_hw(inputs, trace=False)
    return loss


# revision 22
# speedup vs baseline: 1.0620x; 1.0620x over previous
"""DetConB loss (nn_DetConBLoss) on 8 TRN2 NeuronCores via Bass/Tile.

Strategy (data-parallel over batch, targets replicated):
  - Host: l2-normalize preds/targets in f32, flatten to (4096, 256),
    transpose to (d, rows), cast fp8. Core c owns pred rows
    [c*512, (c+1)*512). Each core receives the full targets with columns
    rolled by c*512 so its own-image diagonal band sits at a fixed,
    compile-time-constant column range (the program is SPMD-identical).
  - Device (per core), per (pred, target) combination u of 4:
    * NORMAL part, target cols [0, CN): per 128-row tile, fp8 DoubleRow
      matmuls into PSUM; ScalarE exp (fused scale) with the ACT
      accumulator producing row sums for free.
    * TRANSPOSED part, target cols [CN, 4096): blocks of 128 target
      cols become the PSUM partition dim (lhsT = target slice, rhs =
      all 512 preds). Each block's exp lands in SBUF as fp8 bit
      patterns: DVE blocks via a Schraudolph uint8 construct
      (y = x*SA8 + SB8, bitcast fp8e4), ACT blocks via exp with fp8e4
      output. A ones-vector DoubleRow matmul then contracts each block
      pair over its 256 target partitions, accumulating per-pred
      denominator partials in PSUM — the reduction runs on the
      TensorE, freeing both vector engines.
    Only ~40 KB of row-sum partials leave the device.
  - Host: the 16x16 own-image diagonal dot blocks (recomputed from the
    same fp8 inputs), masks from the roi indices, positive-pair sums,
    the -inf masking correction, log, and the final mean.
"""
import numpy as np
import ml_dtypes

import concourse.bacc as bacc
import concourse.mybir as mybir
import concourse.tile as tile
from concourse.bass_utils import run_bass_kernel_spmd

TEMP = 0.1
EPS = 1e-11
SCALE = float(np.float32(1.0 / (TEMP + EPS)))
NCORES = 8
B, N, D = 256, 16, 256
R = B * N          # 4096 flat rows
RPC = R // NCORES  # 512 rows per core
MT = RPC // 128    # 4 row-tiles of 128 per core
BF16 = mybir.dt.bfloat16
FP8 = mybir.dt.float8e4
U8 = mybir.dt.uint8
NPFP8 = ml_dtypes.float8_e4m3
F32 = mybir.dt.float32
I32 = mybir.dt.int32

NPAIR = R // 256             # 16 transposed 256-target-column pairs per combo


def is_act_pair(px, pair):
    """Consumer engine for (combo px, pair): alternates so each interleaved
    slot pair has one ScalarE and one DVE consumer."""
    return pair % 2 == 1

# Schraudolph fast-exp, f32 flavor (normal part no longer uses it; kept
# for reference/testing) and e4m3 flavor for the transposed DVE blocks:
# exp(x) ~= bitcast_fp8e4(uint8(x*SA8 + SB8)), x = scale*logit.
SA8 = float(np.float32(8.0 / np.log(2.0)))
SB8 = float(np.float32(7 * 8 - 486411.0 / 2**20))


def build_nc():
    """Build + schedule + compile the SPMD per-core Bass program."""
    nc = bacc.Bacc("TRN2", target_bir_lowering=False, debug=False,
                   num_devices=NCORES)

    # p layout [128, 2048]: k0 at cols [0,512), k1 at [1024,1536), rest pad
    # (the 512-byte gap keeps the DoubleRow k-pair fetch off a single SBUF
    # line; adjacent k-runs halve the PE's fp8 stream rate).
    p_dram = [nc.dram_tensor(f"p{i + 1}t", [128, 4 * RPC], FP8,
                             kind="ExternalInput") for i in range(2)]
    t_dram = [nc.dram_tensor(f"t{i + 1}t", [D, R], FP8, kind="ExternalInput")
              for i in range(2)]
    dsum = nc.dram_tensor("dsum", [2, RPC], F32, kind="ExternalOutput")

    with tile.TileContext(nc) as tc:
        with (
            tc.tile_pool(name="const", bufs=1) as const_pool,
            tc.tile_pool(name="psum", bufs=1, space="PSUM") as psum_pool,
            tc.tile_pool(name="scratch", bufs=2) as scratch_pool,
        ):
            t_sb = [const_pool.tile([128, 2 * R], FP8, name=f"t_sb{i}", tag=f"t{i}")
                    for i in range(2)]
            p_sb = [const_pool.tile([128, 4 * RPC], FP8, name=f"p_sb{i}", tag=f"p{i}")
                    for i in range(2)]

            warm = const_pool.tile([128, 2], F32, name="warm", tag="warm")
            zbias = const_pool.tile([128, 1], F32, name="zbias", tag="zbias")
            ones8 = const_pool.tile([128, 64], FP8, name="ones8", tag="ones8")

            # Input DMAs. The three loads gating the first matmul (both k
            # halves of t1's first 512 cols + p1) are spread across THREE
            # descriptor-generation engines — gpsimd (SWDGE), Activation,
            # and sync — and emitted before everything else so each engine's
            # earliest cycles go to them.
            def load_t_cols(tsel, k, c0, c1, eng=None):
                (eng or nc.sync).dma_start(
                    out=t_sb[tsel][:, k * R + c0: k * R + c1],
                    in_=t_dram[tsel][k * 128:(k + 1) * 128, c0:c1])

            def load_t(tsel, k, g):
                load_t_cols(tsel, k, g * 2048, (g + 1) * 2048)

            def load_p(px, eng):
                # only the two used 512-col halves, not the padding
                for k in range(2):
                    eng.dma_start(
                        out=p_sb[px][:, k * 2 * RPC:k * 2 * RPC + RPC],
                        in_=p_dram[px].ap()[:, k * 2 * RPC:k * 2 * RPC + RPC])

            load_t_cols(0, 0, 0, 512, eng=nc.gpsimd)
            load_t_cols(0, 1, 0, 512, eng=nc.scalar)
            load_p(0, nc.sync)
            load_p(1, nc.scalar)
            load_t_cols(0, 0, 512, 2048)
            load_t_cols(0, 1, 512, 2048)
            load_t(0, 0, 1)
            load_t(0, 1, 1)

            nc.vector.memset(warm, 0.0)
            # Explicit zero-bias AP: a float bias would be lowered through the
            # const-AP machinery, whose TENSOR_LOAD sits in the preamble.
            nc.vector.memset(zbias, 0.0)
            # 32 identical ones columns: a DoubleRow LDWEIGHTS with a single
            # weight column fails the compiler's ISA check, so the sum
            # matmul produces 32 duplicate rows (only row 0 is read).
            nc.vector.memset(ones8, 1.0)
            # Warm the exp table set during the input-DMA window so the first
            # real ACTIVATE does not pay the ~2.7us ACT_TABLE_LOAD.
            nc.scalar.activation(warm, warm,
                                 mybir.ActivationFunctionType.Exp, bias=zbias)

            rhs3 = [t_sb[i].rearrange("p (k c) -> p k c", k=2) for i in range(2)]
            lhs3 = [p_sb[i].rearrange("p (k c) -> p k c", k=2) for i in range(2)]
            onesT = ones8.rearrange("p (k m) -> p k m", k=2)
            dummy = const_pool.tile([128, 2048], FP8, name="dummy",
                                    tag="dummy")
            nc.vector.memset(dummy, 0.0)
            dummy3 = dummy.rearrange("p (k c) -> p k c", k=2)

            # Per-pred denominator partials: combo u accumulates into
            # partitions [0,32), column half u%2, of one persistent tile
            # (combo u+2 reuses the half after u's row is copied out).
            psum_acc = psum_pool.tile([128, 2 * RPC], F32, name="psum_acc",
                                      tag="pss")
            dsb = const_pool.tile([128, 2 * RPC], F32, name="dsb", tag="dsb")

            def mm2(tsel, px, pair, sch):
                # ones-DoubleRow matmul: contract the pair's 256 target
                # partitions; accumulate per-pred sums in px's column half.
                # The loss only uses S[p1] = S[0]+S[2] and S[p2] = S[1]+S[3],
                # so both tsel combos of a px share one accumulation group.
                nc.tensor.matmul(
                    psum_acc[0:32, px * RPC:(px + 1) * RPC],
                    onesT,
                    sch.bitcast(FP8).rearrange(
                        "p (k c) -> p k c", k=2)[:, :, 0:RPC],
                    start=(pair == 0 and tsel == 0),
                    stop=(pair == NPAIR - 1 and tsel == 1),
                    perf_mode=mybir.MatmulPerfMode.DoubleRow)

            # The MM2 pending queue is global: a combo's last sum matmuls
            # flush interleaved into the NEXT combo's matmul stream (the two
            # psum_acc halves don't conflict), so the PE never drains at a
            # combo boundary. Each combo's finished row is bounced to SBUF
            # (PSUM is not DMA-readable) and shipped right after its final
            # sum matmul; the copy also frees the column half for combo+2.
            pending = []

            # PE HAM pre-warm: ~3.4us of dummy matmuls during the input-DMA
            # window lift the PE clock gate to 8/8 before the first real
            # matmul, saving the ~2-3us cold tax (cold MMs run 1.2 GHz).
            # They write the psum_acc region, which the first real sum
            # matmul's start=True clears.
            for _ in range(14):
                nc.tensor.matmul(
                    psum_acc[0:32, 0:RPC], onesT, dummy3[:, :, 0:RPC],
                    start=True, stop=True,
                    perf_mode=mybir.MatmulPerfMode.DoubleRow)

            def flush_one():
                combo, px, pair, sch = pending.pop(0)
                tsel = combo // 2
                mm2(tsel, px, pair, sch)
                if pair == NPAIR - 1 and tsel == 1:
                    # px0 closes while combo 3 still computes: its copy
                    # (on the DVE, the less-loaded consumer) overlaps.
                    nc.vector.tensor_scalar(
                        dsb[0:1, px * RPC:(px + 1) * RPC],
                        psum_acc[0:1, px * RPC:(px + 1) * RPC], 0.0, 0.0,
                        op0=mybir.AluOpType.add,
                        op1=mybir.AluOpType.bypass)
                    nc.sync.dma_start(
                        out=dsum.ap()[px:px + 1, :],
                        in_=dsb[0:1, px * RPC:(px + 1) * RPC])

            for tsel in range(2):
                for px in range(2):
                    combo = tsel * 2 + px
                    for pair in range(NPAIR):
                        if combo == 0 and pair in (5, 11):
                            # t2 is first needed at combo 2 (~half-way); two
                            # 1-col memset anchors on the DVE queue stagger
                            # its 1 MB of DMA traffic through combo 0, whose
                            # matmuls otherwise lose fp8 fetch bandwidth to
                            # the incoming stream (Q0 MMs 486ns vs 380).
                            g = 0 if pair == 5 else 1
                            for k in range(2):
                                nc.vector.memset(
                                    t_sb[1][:, k * R + g * 2048:
                                            k * R + g * 2048 + 1], 0.0)
                            for k in range(2):
                                load_t(1, k, g)
                        # pair = 256 target cols as PSUM partitions: two
                        # 128-col DoubleRow matmuls into one 2-bank tile.
                        psT = psum_pool.tile([128, 2 * RPC], F32,
                                             name="psT", tag="pst", bufs=3)
                        for half in range(2):
                            tc0 = (2 * pair + half) * 128
                            nc.tensor.matmul(
                                psT[:, half * RPC:(half + 1) * RPC],
                                rhs3[tsel][:, :, tc0:tc0 + 128],
                                lhs3[px][:, :, 0:RPC],
                                start=True, stop=True,
                                perf_mode=mybir.MatmulPerfMode.DoubleRow)
                        # The pair's sum matmul is emitted three pairs later
                        # so the in-order PE queue never head-blocks waiting
                        # for the exp data.
                        if len(pending) == 3:
                            flush_one()
                        # One fused consumer per pair turns both halves into
                        # fp8e4 exp bit patterns in the gapped sch slots
                        # (k-runs 1 KB apart keep the PE fetch at full rate).
                        sch = scratch_pool.tile([128, 2048], U8,
                                                name="sch", tag="sch", bufs=6)
                        pin = psT.rearrange("p (k c) -> p k c", k=2)
                        pout = sch.rearrange("p (k c) -> p k c", k=2)[:, :, 0:RPC]
                        if is_act_pair(px, pair):
                            nc.scalar.activation(
                                pout.bitcast(FP8), pin,
                                mybir.ActivationFunctionType.Exp,
                                bias=zbias, scale=SCALE)
                        else:
                            nc.vector.tensor_scalar(
                                pout, pin, SA8 * SCALE, SB8,
                                op0=mybir.AluOpType.mult,
                                op1=mybir.AluOpType.add)
                        pending.append((combo, px, pair, sch))
            while pending:
                flush_one()

    nc.compile()
    return nc


_NC = None


def _get_nc():
    global _NC
    if _NC is None:
        _NC = build_nc()
    return _NC


def _l2norm(x):
    return x / np.linalg.norm(x, axis=-1, keepdims=True)


def _dev_p_layout(pt):
    # pt: [D=256, RPC] fp8 -> [128, 2048] with k0 at [0,512), k1 at [1024,1536)
    out = np.zeros((128, 4 * RPC), NPFP8)
    out[:, 0:RPC] = pt[0:128]
    out[:, 2 * RPC:3 * RPC] = pt[128:256]
    return out


def host_prep(pred1, pred2, target1, target2):
    p1t = _l2norm(np.asarray(pred1, np.float32)).reshape(R, D).T.astype(NPFP8)
    p2t = _l2norm(np.asarray(pred2, np.float32)).reshape(R, D).T.astype(NPFP8)
    t1t = _l2norm(np.asarray(target1, np.float32)).reshape(R, D).T.astype(NPFP8)
    t2t = _l2norm(np.asarray(target2, np.float32)).reshape(R, D).T.astype(NPFP8)
    # Raw own-image diagonal dot blocks (b, n, m), fp8-quantized operands in
    # f32 — the same products the device computes, ~0.4% of total FLOPs.
    pf = [p1t.T.astype(np.float32).reshape(B, N, D),
          p2t.T.astype(np.float32).reshape(B, N, D)]
    tf = [t1t.T.astype(np.float32).reshape(B, N, D),
          t2t.T.astype(np.float32).reshape(B, N, D)]
    diag = [[np.einsum('bnd,bmd->bnm', pf[px], tf[ts]).astype(np.float32)
             for ts in range(2)] for px in range(2)]
    in_maps = []
    for c in range(NCORES):
        r0 = c * RPC
        in_maps.append({
            "p1t": _dev_p_layout(p1t[:, r0:r0 + RPC]),
            "p2t": _dev_p_layout(p2t[:, r0:r0 + RPC]),
            "t1t": np.ascontiguousarray(np.concatenate([t1t[:, r0:], t1t[:, :r0]], axis=1)),
            "t2t": np.ascontiguousarray(np.concatenate([t2t[:, r0:], t2t[:, :r0]], axis=1)),
        })
    return in_maps, diag


def host_post(results, diag, pind1, pind2, tind1, tind2):
    S = np.zeros((2, R), np.float64)
    for c, res in enumerate(results):
        dsumv = np.asarray(res["dsum"]).astype(np.float64)
        for px in range(2):
            r0 = c * RPC
            S[px, r0:r0 + RPC] = dsumv[px]
    sc = np.float32(SCALE)
    D_aa = sc * diag[0][0]
    D_ab = sc * diag[0][1]
    D_ba = sc * diag[1][0]
    D_bb = sc * diag[1][1]

    f32 = np.float32
    pind1, pind2 = np.asarray(pind1), np.asarray(pind2)
    tind1, tind2 = np.asarray(tind1), np.asarray(tind2)
    same_aa = (pind1[:, :, None] == tind1[:, None, :]).astype(f32)
    same_ab = (pind1[:, :, None] == tind2[:, None, :]).astype(f32)
    same_ba = (pind2[:, :, None] == tind1[:, None, :]).astype(f32)
    same_bb = (pind2[:, :, None] == tind2[:, None, :]).astype(f32)

    S0 = S[0].reshape(B, N)
    S1 = S[1].reshape(B, N)
    corr0 = (same_aa * np.exp(D_aa.astype(np.float64))).sum(-1)
    corr1 = (same_bb * np.exp(D_bb.astype(np.float64))).sum(-1)
    lse0 = np.log(S0 - corr0)
    lse1 = np.log(S1 - corr1)

    num_pos0 = same_ab.sum(-1)
    num_pos1 = same_ba.sum(-1)
    pos_sum0 = (same_ab * D_ab).sum(-1)
    pos_sum1 = (same_ba * D_ba).sum(-1)

    area0 = (pind1[:, :, None] == pind1[:, None, :]).astype(f32).sum(-1)
    area1 = (pind2[:, :, None] == pind2[:, None, :]).astype(f32).sum(-1)
    w0 = (num_pos0 > 0.001).astype(f32) / area0
    w1 = (num_pos1 > 0.001).astype(f32) / area1

    ce0 = -w0 * (pos_sum0 - num_pos0 * lse0) / np.maximum(num_pos0, 1.0)
    ce1 = -w1 * (pos_sum1 - num_pos1 * lse1) / np.maximum(num_pos1, 1.0)
    return np.float32(ce0.mean() + ce1.mean())


def run_hw(inputs, trace=False):
    nc = _get_nc()
    in_maps, diag = host_prep(inputs["pred1"], inputs["pred2"],
                              inputs["target1"], inputs["target2"])
    last_err = None
    for attempt in range(3):
        try:
            res = run_bass_kernel_spmd(nc, in_maps,
                                       core_ids=list(range(NCORES)),
                                       trace=trace)
            break
        except Exception as e:  # transient NRT device errors recover on retry
            last_err = e
            import time
            time.sleep(20 * (attempt + 1))
    else:
        raise last_err
    loss = host_post(res.results, diag, inputs["pind1"], inputs["pind2"],
                     inputs["tind1"], inputs["tind2"])
    return loss, res


def kernel(**inputs):
    loss, _ = run_hw(inputs, trace=False)
    return loss
